# revision 69
# baseline (speedup 1.0000x reference)
"""Trainium2 Bass kernel for CARE position encoding (rotor sandwich product).

The reference computes out = R x R~ where R is a product of 4 plane rotors
(cos(phi_i) + sin(phi_i) e_mi) with phi_i = 0.5 * c_i * theta[pos, i].
Algebraically this factorizes into 4 sequential Givens-rotation stages: for
plane bivector e_m, the 8 basis blades A with |A & m| == 1 rotate in 4
disjoint pairs (A, A^m) by angle 2*phi with pair signs tau = C[A, m, A^m];
the other 8 blades pass through unchanged:
    out[a] = c2*x[a] + tau*s2*x[b] ;  out[b] = c2*x[b] - tau*s2*x[a]

Implementation (data-parallel across 8 cores, batch-sharded, 2 rows/core):
 - component-PLANAR fp16 SBUF layout: the host reorders x per core to
   [P, 16 slots, J] where slot pairs (2p, 2p+1) hold dual blades (A, ~A).
   Duality preserves |A & m| parity, so every plane's 8 rotating blades are
   4 whole slot pairs, and a fixed pair->slot assignment makes each plane's
   T (cos part), U (sin part) and ADD a SINGLE strided DVE op whose
   innermost dim is a contiguous, 4B-aligned run of positions -- which lets
   every fp16 tensor_tensor run in the DVE's packed 2x mode.
 - angles once per core in fp32: A_i = float(pos)*(f_i*c_i) and the
   magic-round on ScalarE, Cody-Waite cascade on DVE, then ScalarE Sin
   emits fp16 tables: a duplicated cos block [c,c] (arg = pi/2 - |R|, which
   stays inside Sin's domain) and a per-plane sign-sequenced sin region
   whose block index realizes every plane's pair-sign pattern affinely.
 - every plane's index arithmetic is verified symbolically against the
   input Cayley tensor at kernel-build time.
"""
import contextlib

import numpy as np

import concourse.bass as bass
import concourse.tile as tile
from concourse import bacc, mybir
from concourse.bass_utils import run_bass_kernel_spmd

F32 = mybir.dt.float32
F16 = mybir.dt.float16
I32 = mybir.dt.int32

P = 128
NCORES = 8
B, L, MV = 16, 16384, 16
MAX_LEN = 16384
ROWS_PER_CORE = B // NCORES          # 2
N = ROWS_PER_CORE * L                # 32768 positions per core
J = N // P                           # 256 positions per partition

PLANE_BLADES = (3, 5, 9, 6)
STAGE_ORDER = (6, 9, 5, 3)           # sandwich applies the last rotor first

# Dual slot pairing: pair p holds blades (first, second) = (A, 15^A) at
# slot planes (2p, 2p+1).  Pair classes (by which planes rotate them) and
# the within-pair order are chosen so each plane's rotating pairs and
# their XOR-partner map are affine in <=2 AP dims (see _PLANE_OPS).
PAIRS = ((6, 9), (2, 13), (1, 14), (5, 10), (3, 12), (8, 7), (4, 11), (0, 15))
COMP_OF_SLOT = tuple(c for pr in PAIRS for c in pr)
SLOT_OF_COMP = tuple(COMP_OF_SLOT.index(c) for c in range(16))

MAGIC = float(np.float32(1.5 * 2 ** 23))
TWO_PI = 2.0 * np.pi
INV_2PI = float(np.float32(1.0 / TWO_PI))
HALF_PI = float(np.float32(np.pi / 2.0))
PI_F = float(np.float32(np.pi))
TWO_PI_F = float(np.float32(TWO_PI))

# per-plane SSX block sign sequences (block b holds PLANE_SEQ[m][b] * sin)
_PLANE_SEQ = {3: (1, -1, 1), 5: (1, 1, -1, -1), 9: (1, -1, 1), 6: (1, -1, 1)}


def _cw_split(val, bits=12):
    def trunc(v):
        u = np.float32(v).view(np.uint32)
        u = np.uint32(u & np.uint32((0xFFFFFFFF << (24 - bits)) & 0xFFFFFFFF))
        return u.view(np.float32)
    c1 = trunc(val)
    c2 = trunc(val - np.float64(c1))
    c3 = np.float32(val - np.float64(c1) - np.float64(c2))
    return float(c1), float(c2), float(c3)


CW1, CW2, CW3 = _cw_split(TWO_PI)

# Per-plane op descriptors in SLOT-PLANE units (strides/offsets are
# multiples of J elements).  Each sub:
#   x0/xd : slot-plane offset / [step,count] dims of the X read (T,U) or
#           X write (ADD, which reuses the T sub's x side)
#   t0/td : offset / dims in the T/U tile (8 slot planes, pair-major)
#   s0/sd (U only): SSX block offset / per-dim block steps
# The AP builder appends the position dim [1, J]; when every operand's
# innermost slot dim is [1, 2] (adjacent slot planes / adjacent blocks) it
# is merged with the position dim into a contiguous [1, 2J] run.
_PLANE_OPS = {
    3: dict(  # pairs (G,B,A,F) = slots 0..3; partners reversed.  ADD is
        # split 3 pairs + 1 + 1 slots so each part's result can feed an
        # early output DMA, with the last chunks kept small for the tail.
        t=[dict(x0=0, xd=[[4, 2], [2, 2], [1, 2]],
                t0=0, td=[[4, 2], [2, 2], [1, 2]])],
        u=[dict(x0=6, xd=[[-4, 2], [-2, 2], [1, 2]],
                t0=0, td=[[4, 2], [2, 2], [1, 2]],
                s0=1, sd=[[-1, 2], [0, 2], [1, 2]])],
        a=[dict(x0=0, xd=[[2, 3], [1, 2]], t0=0, td=[[2, 3], [1, 2]]),
           dict(x0=6, xd=[[1, 2]], t0=6, td=[[1, 2]])],
    ),
    5: dict(  # pairs (G,A,E,C) = slots {0,2,4,6}; partners = +4 mod 8
        t=[dict(x0=0, xd=[[4, 4], [1, 2]],
                t0=0, td=[[2, 4], [1, 2]])],
        u=[dict(x0=8, xd=[[-8, 2], [4, 2], [1, 2]],
                t0=0, td=[[4, 2], [2, 2], [1, 2]],
                s0=0, sd=[[2, 2], [0, 2], [1, 2]])],
    ),
    9: dict(  # pairs (A,F,E,D) = slots 2..5; A<->D keep order, F<->E flip
        t=[dict(x0=4, xd=[[2, 4], [1, 2]],
                t0=0, td=[[2, 4], [1, 2]])],
        u=[dict(x0=10, xd=[[-6, 2], [1, 2]],
                t0=0, td=[[6, 2], [1, 2]],
                s0=0, sd=[[1, 2], [1, 2]]),
           dict(x0=9, xd=[[-2, 2], [-1, 2]],
                t0=2, td=[[2, 2], [1, 2]],
                s0=1, sd=[[0, 2], [-1, 2]])],
    ),
    6: dict(  # pairs (B,F,E,C) = slots {1,3,4,6}; partner = 6-3k1-2k2
        t=[dict(x0=2, xd=[[6, 2], [4, 2], [1, 2]],
                t0=0, td=[[4, 2], [2, 2], [1, 2]])],
        u=[dict(x0=12, xd=[[-6, 2], [-4, 2], [1, 2]],
                t0=0, td=[[4, 2], [2, 2], [1, 2]],
                s0=0, sd=[[0, 2], [1, 2], [1, 2]])],
    ),
}


def _iter_idx(dims):
    import itertools
    return itertools.product(*[range(c) for (_, c) in dims])


def _off(dims, idx):
    return sum(s * i for (s, _), i in zip(dims, idx))


def _verify_plane_ops(cayley):
    """Symbolically apply the descriptor index arithmetic for one position:
    out[comp] = c2*x[tcomp] + seqsign*s2*x[ucomp], and check it equals the
    Cayley-derived Givens stage for every plane.  Raises on mismatch."""
    for m in PLANE_BLADES:
        ops = _PLANE_OPS[m]
        tmap, umap, usgn, amap = {}, {}, {}, {}
        for sub in ops["t"]:
            for idx in _iter_idx(sub["xd"]):
                tp = sub["t0"] + _off(sub["td"], idx)
                sp = sub["x0"] + _off(sub["xd"], idx)
                assert tp not in tmap, (m, tp)
                tmap[tp] = sp
        for sub in ops.get("a", ops["t"]):
            for idx in _iter_idx(sub["xd"]):
                tp = sub["t0"] + _off(sub["td"], idx)
                sp = sub["x0"] + _off(sub["xd"], idx)
                assert tp not in amap, (m, tp)
                amap[tp] = sp
        assert amap == tmap, (m, amap, tmap)
        seq = _PLANE_SEQ[m]
        for sub in ops["u"]:
            for idx in _iter_idx(sub["xd"]):
                tp = sub["t0"] + _off(sub["td"], idx)
                sp = sub["x0"] + _off(sub["xd"], idx)
                blk = sub["s0"] + _off(list(zip([s for s, _ in sub["sd"]],
                                                [c for _, c in sub["xd"]])), idx)
                assert 0 <= blk < len(seq), (m, blk)
                assert tp not in umap, (m, tp)
                umap[tp] = sp
                usgn[tp] = seq[blk]
        assert sorted(tmap) == sorted(umap) == list(range(8)), m
        for tp in range(8):
            a = COMP_OF_SLOT[tmap[tp]]
            b = a ^ m
            assert COMP_OF_SLOT[umap[tp]] == b, (m, tp, COMP_OF_SLOT[umap[tp]], b)
            tau = float(cayley[a, m, b])
            assert usgn[tp] == tau, (m, tp, usgn[tp], tau)
        # every rotating slot pair must be read/written exactly once
        rot = sorted(tmap.values())
        expect = sorted(s for s in range(16)
                        if bin(COMP_OF_SLOT[s] & m).count("1") == 1)
        assert rot == expect, (m, rot, expect)


def _ap_with_dims(base_ap, extra_off, dims):
    ap = [list(base_ap.ap[0])] + [list(d) for d in dims]
    return bass.AP(base_ap.tensor, base_ap.offset + extra_off, ap)


def _merged(sub):
    md = all(d[-1] == [1, 2] for d in (sub["xd"], sub["td"]))
    if "sd" in sub:
        md = md and sub["sd"][-1][0] == 1
    return md


def _el(slotdims, merged):
    if merged:
        return [[s * J, n] for s, n in slotdims[:-1]] + [[1, 2 * J]]
    return [[s * J, n] for s, n in slotdims] + [[1, J]]


def _el_blk(blksteps, counts, merged):
    if merged:
        return [[s * J, n] for (s, _), (_, n) in
                zip(blksteps[:-1], counts[:-1])] + [[1, 2 * J]]
    return [[s * J, n] for (s, _), (_, n) in zip(blksteps, counts)] + [[1, J]]


def _cdims(sub, merged):
    if merged:
        return [[0, n] for _, n in sub["td"][:-1]] + [[1, 2 * J]]
    return [[0, n] for _, n in sub["td"]] + [[1, J]]


def _build_program(freqs, coefs):
    nc = bacc.Bacc("TRN2", target_bir_lowering=False, debug=False,
                   enable_asserts=False, num_devices=NCORES)
    x_d = nc.dram_tensor("x", [P, 16 * J], F16, kind="ExternalInput")
    pos_d = nc.dram_tensor("pos", [P, J], I32, kind="ExternalInput")
    out_d = nc.dram_tensor("out", [P, 16 * J], F16, kind="ExternalOutput")

    SIN = mybir.ActivationFunctionType.Sin
    plane_i = {m: PLANE_BLADES.index(m) for m in STAGE_ORDER}

    with tile.TileContext(nc) as tc:
        with tc.tile_pool(name="const", bufs=1) as cpool, \
             tc.tile_pool(name="ang", bufs=2) as apool, \
             tc.tile_pool(name="tmp", bufs=2) as tpool:

            X = cpool.tile([P, 16 * J], F16)
            Pp = cpool.tile([P, J], I32)
            nc.sync.dma_start(Pp[:], pos_d[:])
            # the slots stage m=6 reads (its strided APs span [2J,14J)
            # conservatively).  Two wide chunks beat three exact ones: fewer
            # queue armings and 4KB-row descriptors keep the DMA engines
            # rate-efficient, even though [4J,6J) isn't needed until stage 2.
            nc.sync.dma_start(X[:, 2 * J:10 * J], x_d[:, 2 * J:10 * J])
            nc.sync.dma_start(X[:, 12 * J:14 * J], x_d[:, 12 * J:14 * J])

            # ---- per-plane angle pipelines + fp16 sin/cos tables ----
            # A = pos*(f*c) and K = round(A/2pi) on ScalarE (scale/bias folds
            # the magic-round); only the Cody-Waite cascade runs on DVE.
            COPY = mybir.ActivationFunctionType.Copy
            IDENT = mybir.ActivationFunctionType.Identity
            ABS = mybir.ActivationFunctionType.Abs
            CB = {}
            for ci, v in enumerate((MAGIC, -MAGIC, HALF_PI)):
                cb = cpool.tile([P, 1], F32, name=f"bias{ci}")
                nc.gpsimd.memset(cb[:], v)
                CB[v] = cb
            # dummy Sin so the one act-table set holding Copy+Identity+Sin
            # (trig_and_small) is loaded ONCE, before the angle chains --
            # otherwise the first Copy loads a sin-less set and the first
            # real Sin pays a 1.3us table reload on the critical path.
            warm = cpool.tile([P, 1], F32, name="warm")
            nc.scalar.activation(warm[:], CB[HALF_PI][:], SIN)
            Cd, SX = {}, {}
            for si, m in enumerate(STAGE_ORDER):
                i = plane_i[m]
                fc = float(np.float32(np.float32(freqs[i]) *
                                      np.float32(coefs[i])))
                A = apool.tile([P, J], F32, tag="a")
                K1 = apool.tile([P, J], F32, tag="k1")
                K = apool.tile([P, J], F32, tag="k")
                R = apool.tile([P, J], F32, tag="r")
                RC = apool.tile([P, J], F32, tag="rc")
                # A = pos*(f*c) and K = round(A/2pi) on ScalarE (scale/bias
                # folds the magic round); Cody-Waite on DVE.  The first
                # plane's chain gates the whole stage pipeline: pin it to
                # the front of every engine queue.
                def prio(first=si == 0):
                    return (tc.high_priority() if first
                            else contextlib.nullcontext())
                with prio():
                    if si == 0:
                        # K and |R| on the (idle) DVE so the ScalarE queue
                        # never stalls this chain behind later planes' ops
                        nc.scalar.activation(K1[:], Pp[:], IDENT,
                                             bias=CB[MAGIC][:],
                                             scale=float(np.float32(fc) *
                                                         INV_2PI))
                        nc.scalar.activation(A[:], Pp[:], COPY, scale=fc)
                        nc.vector.tensor_scalar_add(K[:], K1[:], -MAGIC)
                        nc.vector.cody_waite_cascade(R[:], A[:], K[:],
                                                     CW1, CW2, CW3)
                        # cos arg via the one-op wrap on the still-idle DVE:
                        # Cd then has no ScalarE prerequisite, so it can't
                        # lose its queue slot to the (ready-earlier) SX sins
                        nc.vector.add_range_wrap(RC[:], R[:], HALF_PI,
                                                 PI_F, TWO_PI_F)
                    else:
                        nc.scalar.activation(A[:], Pp[:], COPY, scale=fc)
                        nc.scalar.activation(K1[:], Pp[:], IDENT,
                                             bias=CB[MAGIC][:],
                                             scale=float(np.float32(fc) *
                                                         INV_2PI))
                        if si == 1:
                            # m=9's K/CW fit in the DVE's pre-stage idle gap
                            nc.vector.tensor_scalar_add(K[:], K1[:], -MAGIC)
                        else:
                            # m=5/m=3's chains resolve mid-window: keep their
                            # K on ScalarE so the DVE only runs the cascade
                            nc.scalar.activation(K[:], K1[:], IDENT,
                                                 bias=CB[-MAGIC][:])
                        nc.vector.cody_waite_cascade(R[:], A[:], K[:],
                                                     CW1, CW2, CW3)

                seq = _PLANE_SEQ[m]
                Cd[m] = cpool.tile([P, 2 * J], F16, name=f"cd{m}",
                                   tag=f"c{m}")
                SX[m] = cpool.tile([P, len(seq) * J], F16, name=f"sx{m}",
                                   tag=f"s{m}")
                with prio():
                    if si == 0:
                        nc.scalar.activation(
                            _ap_with_dims(Cd[m][:], 0, [[J, 2], [1, J]]),
                            _ap_with_dims(RC[:], 0, [[0, 2], [1, J]]), SIN)
                    else:
                        # cos arg as sin(pi/2 - |R|): stays in Sin's domain
                        nc.scalar.activation(RC[:], R[:], ABS)
                        nc.scalar.activation(
                            _ap_with_dims(Cd[m][:], 0, [[J, 2], [1, J]]),
                            _ap_with_dims(RC[:], 0, [[0, 2], [1, J]]), SIN,
                            scale=-1.0, bias=CB[HALF_PI][:])
                # the sin tables gate only the U op (one T later than Cd):
                # left outside the priority block so their completion sems
                # don't batch with Cd's and stall the first T
                for sgn in (1.0, -1.0):
                    blks = [b for b, s in enumerate(seq) if s == sgn]
                    if len(blks) == 1:
                        od = [[1, J]]
                    else:
                        od = [[(blks[1] - blks[0]) * J, len(blks)],
                              [1, J]]
                    nc.scalar.activation(
                        _ap_with_dims(SX[m][:], blks[0] * J, od),
                        _ap_with_dims(R[:], 0,
                                      [[0, len(blks)], [1, J]][-len(od):]),
                        SIN, scale=sgn)

            # slots 10,11 (first needed by the 2nd stage), slots 0,1 (3rd
            # stage) and the slots-14,15 DRAM->DRAM passthrough (never
            # rotated).  Emitted after the angle chains so their completion
            # sems don't batch with the critical DMAs'.
            nc.sync.dma_start(X[:, 10 * J:12 * J], x_d[:, 10 * J:12 * J])
            nc.sync.dma_start(X[:, :2 * J], x_d[:, :2 * J])
            nc.sync.dma_start(out_d[:, 14 * J:], x_d[:, 14 * J:])

            # ---- Givens stages (innermost rotor first) ----
            for si, m in enumerate(STAGE_ORDER):
                ops = _PLANE_OPS[m]
                T = tpool.tile([P, 8 * J], F16, tag="t")
                U = tpool.tile([P, 8 * J], F16, tag="u")
                for sub in ops["t"]:
                    md = _merged(sub)
                    nc.vector.tensor_mul(
                        _ap_with_dims(T[:], sub["t0"] * J, _el(sub["td"], md)),
                        _ap_with_dims(X[:], sub["x0"] * J, _el(sub["xd"], md)),
                        _ap_with_dims(Cd[m][:], 0, _cdims(sub, md)))
                for sub in ops["u"]:
                    md = _merged(sub)
                    nc.vector.tensor_mul(
                        _ap_with_dims(U[:], sub["t0"] * J, _el(sub["td"], md)),
                        _ap_with_dims(X[:], sub["x0"] * J, _el(sub["xd"], md)),
                        _ap_with_dims(SX[m][:], sub["s0"] * J,
                                      _el_blk(sub["sd"], sub["xd"], md)))
                for sub in ops.get("a", ops["t"]):
                    md = _merged(sub)
                    nc.vector.tensor_add(
                        _ap_with_dims(X[:], sub["x0"] * J, _el(sub["xd"], md)),
                        _ap_with_dims(T[:], sub["t0"] * J, _el(sub["td"], md)),
                        _ap_with_dims(U[:], sub["t0"] * J, _el(sub["td"], md)))
                if si == 1:
                    # pair D (slots 10,11) is final after the 2nd stage (m=9)
                    nc.sync.dma_start(out_d[:, 10 * J:12 * J],
                                      X[:, 10 * J:12 * J])
                if si == 2:
                    # pairs E,C (slots 8,9 / 12,13) final after 3rd stage
                    nc.sync.dma_start(out_d[:, 8 * J:10 * J],
                                      X[:, 8 * J:10 * J])
                    nc.sync.dma_start(out_d[:, 12 * J:14 * J],
                                      X[:, 12 * J:14 * J])
            # final stage (m=3) ADD is split 3+1 pairs; each part's slots DMA
            # out as soon as its ADD lands.  Two chunks, not three: every
            # extra trigger costs ~0.6us serially on the sync sequencer at
            # the very tail of the kernel.
            nc.sync.dma_start(out_d[:, :6 * J], X[:, :6 * J])
            nc.sync.dma_start(out_d[:, 6 * J:8 * J], X[:, 6 * J:8 * J])

    nc.compile()
    return nc


_PROGRAM_CACHE = {}


def _get_program(freqs, coefs):
    key = (tuple(freqs), tuple(coefs))
    if key not in _PROGRAM_CACHE:
        _PROGRAM_CACHE[key] = _build_program(freqs, coefs)
    return _PROGRAM_CACHE[key]


def _derive_params(inputs):
    coefs = [float(np.asarray(inputs[c], dtype=np.float32).reshape(MV)[b])
             for c, b in zip(("bx", "by", "bz", "bw"), PLANE_BLADES)]
    theta = np.asarray(inputs["theta"], dtype=np.float32)
    freqs = [float(theta.reshape(MAX_LEN, 4)[1, i]) for i in range(4)]
    return freqs, coefs


def _core_input(x, pos, g):
    xg = np.asarray(x[g * ROWS_PER_CORE:(g + 1) * ROWS_PER_CORE],
                    dtype=np.float32).reshape(P, J, MV)
    planar = xg[:, :, COMP_OF_SLOT].transpose(0, 2, 1)
    pg = np.clip(pos[g * ROWS_PER_CORE:(g + 1) * ROWS_PER_CORE],
                 0, MAX_LEN - 1).astype(np.int32).reshape(P, J)
    return {"x": np.ascontiguousarray(planar.astype(np.float16)
                                      ).reshape(P, 16 * J),
            "pos": np.ascontiguousarray(pg)}


def _core_output(res_g):
    r = np.asarray(res_g).reshape(P, 16, J).transpose(0, 2, 1)
    return r[:, :, SLOT_OF_COMP].astype(np.float32).reshape(
        ROWS_PER_CORE, L, MV)


def kernel(x, pos, bx, by, bz, bw, theta, cayley, biv_mask, scalar_mask):
    x = np.asarray(x, dtype=np.float32)
    pos = np.asarray(pos)
    theta = np.asarray(theta, dtype=np.float32)
    cayley = np.asarray(cayley, dtype=np.float32)

    assert x.shape == (B, L, MV) and pos.shape == (B, L)

    freqs, coefs = _derive_params(
        dict(bx=bx, by=by, bz=bz, bw=bw, theta=theta))
    th_check = np.arange(MAX_LEN, dtype=np.float32)[:, None] * \
        np.asarray(freqs, dtype=np.float32)[None, :]
    assert np.array_equal(th_check, theta.reshape(MAX_LEN, 4)), \
        "theta table is not linear in position; kernel assumption violated"

    _verify_plane_ops(cayley)

    nc = _get_program(freqs, coefs)

    in_maps = [_core_input(x, pos, g) for g in range(NCORES)]
    res = run_bass_kernel_spmd(nc, in_maps, core_ids=list(range(NCORES)))
    out = np.empty((B, L, MV), dtype=np.float32)
    for g in range(NCORES):
        out[g * ROWS_PER_CORE:(g + 1) * ROWS_PER_CORE] = \
            _core_output(res.results[g]["out"])
    return out


# revision 72
# speedup vs baseline: 1.1686x; 1.1686x over previous
"""Trainium2 Bass kernel for CARE position encoding (rotor sandwich product).

The reference computes out = R x R~ where R is a product of 4 plane rotors
(cos(phi_i) + sin(phi_i) e_mi) with phi_i = 0.5 * c_i * theta[pos, i].
Algebraically this factorizes into 4 sequential Givens-rotation stages: for
plane bivector e_m, the 8 basis blades A with |A & m| == 1 rotate in 4
disjoint pairs (A, A^m) by angle 2*phi with pair signs tau = C[A, m, A^m];
the other 8 blades pass through unchanged:
    out[a] = c2*x[a] + tau*s2*x[b] ;  out[b] = c2*x[b] - tau*s2*x[a]

Implementation (data-parallel across 8 cores, batch-sharded, 2 rows/core):
 - component-PLANAR fp16 SBUF layout: the host reorders x per core to
   [P, 16 slots, J] where slot pairs (2p, 2p+1) hold dual blades (A, ~A).
   Duality preserves |A & m| parity, so every plane's 8 rotating blades are
   4 whole slot pairs, and a fixed pair->slot assignment makes each plane's
   T (cos part), U (sin part) and ADD a SINGLE strided DVE op whose
   innermost dim is a contiguous, 4B-aligned run of positions -- which lets
   every fp16 tensor_tensor run in the DVE's packed 2x mode.
 - angles once per core in fp32: A_i = float(pos)*(f_i*c_i) and the
   magic-round on ScalarE, Cody-Waite cascade on DVE, then ScalarE Sin
   emits fp16 tables: a duplicated cos block [c,c] (arg = pi/2 - |R|, which
   stays inside Sin's domain) and a per-plane sign-sequenced sin region
   whose block index realizes every plane's pair-sign pattern affinely.
 - every plane's index arithmetic is verified symbolically against the
   input Cayley tensor at kernel-build time.
"""
import contextlib

import numpy as np

import concourse.bass as bass
import concourse.tile as tile
from concourse import bacc, mybir
from concourse.bass_utils import run_bass_kernel_spmd

F32 = mybir.dt.float32
F16 = mybir.dt.float16
I32 = mybir.dt.int32

P = 128
NCORES = 8
B, L, MV = 16, 16384, 16
MAX_LEN = 16384
ROWS_PER_CORE = B // NCORES          # 2
N = ROWS_PER_CORE * L                # 32768 positions per core
J = N // P                           # 256 positions per partition

PLANE_BLADES = (3, 5, 9, 6)
STAGE_ORDER = (6, 9, 5, 3)           # sandwich applies the last rotor first

# Dual slot pairing: pair p holds blades (first, second) = (A, 15^A) at
# slot planes (2p, 2p+1).  Pair classes (by which planes rotate them) and
# the within-pair order are chosen so each plane's rotating pairs and
# their XOR-partner map are affine in <=2 AP dims (see _PLANE_OPS).
PAIRS = ((6, 9), (2, 13), (1, 14), (5, 10), (3, 12), (8, 7), (4, 11), (0, 15))
COMP_OF_SLOT = tuple(c for pr in PAIRS for c in pr)
SLOT_OF_COMP = tuple(COMP_OF_SLOT.index(c) for c in range(16))

MAGIC = float(np.float32(1.5 * 2 ** 23))
TWO_PI = 2.0 * np.pi
INV_2PI = float(np.float32(1.0 / TWO_PI))
HALF_PI = float(np.float32(np.pi / 2.0))
PI_F = float(np.float32(np.pi))
TWO_PI_F = float(np.float32(TWO_PI))

# per-plane SSX block sign sequences (block b holds PLANE_SEQ[m][b] * sin)
_PLANE_SEQ = {3: (1, -1, 1), 5: (1, 1, -1, -1), 9: (1, -1, 1), 6: (1, -1, 1)}


def _cw_split(val, bits=12):
    def trunc(v):
        u = np.float32(v).view(np.uint32)
        u = np.uint32(u & np.uint32((0xFFFFFFFF << (24 - bits)) & 0xFFFFFFFF))
        return u.view(np.float32)
    c1 = trunc(val)
    c2 = trunc(val - np.float64(c1))
    c3 = np.float32(val - np.float64(c1) - np.float64(c2))
    return float(c1), float(c2), float(c3)


CW1, CW2, CW3 = _cw_split(TWO_PI)

# Per-plane op descriptors in SLOT-PLANE units (strides/offsets are
# multiples of J elements).  Each sub:
#   x0/xd : slot-plane offset / [step,count] dims of the X read (T,U) or
#           X write (ADD, which reuses the T sub's x side)
#   t0/td : offset / dims in the T/U tile (8 slot planes, pair-major)
#   s0/sd (U only): SSX block offset / per-dim block steps
# The AP builder appends the position dim [1, J]; when every operand's
# innermost slot dim is [1, 2] (adjacent slot planes / adjacent blocks) it
# is merged with the position dim into a contiguous [1, 2J] run.
_PLANE_OPS = {
    3: dict(  # pairs (G,B,A,F) = slots 0..3; partners reversed.  ADD is
        # split 3 pairs + 1 + 1 slots so each part's result can feed an
        # early output DMA, with the last chunks kept small for the tail.
        t=[dict(x0=0, xd=[[4, 2], [2, 2], [1, 2]],
                t0=0, td=[[4, 2], [2, 2], [1, 2]])],
        u=[dict(x0=6, xd=[[-4, 2], [-2, 2], [1, 2]],
                t0=0, td=[[4, 2], [2, 2], [1, 2]],
                s0=1, sd=[[-1, 2], [0, 2], [1, 2]])],
        a=[dict(x0=0, xd=[[2, 3], [1, 2]], t0=0, td=[[2, 3], [1, 2]]),
           dict(x0=6, xd=[[1, 2]], t0=6, td=[[1, 2]])],
    ),
    5: dict(  # pairs (G,A,E,C) = slots {0,2,4,6}; partners = +4 mod 8
        t=[dict(x0=0, xd=[[4, 4], [1, 2]],
                t0=0, td=[[2, 4], [1, 2]])],
        u=[dict(x0=8, xd=[[-8, 2], [4, 2], [1, 2]],
                t0=0, td=[[4, 2], [2, 2], [1, 2]],
                s0=0, sd=[[2, 2], [0, 2], [1, 2]])],
    ),
    9: dict(  # pairs (A,F,E,D) = slots 2..5; A<->D keep order, F<->E flip
        t=[dict(x0=4, xd=[[2, 4], [1, 2]],
                t0=0, td=[[2, 4], [1, 2]])],
        u=[dict(x0=10, xd=[[-6, 2], [1, 2]],
                t0=0, td=[[6, 2], [1, 2]],
                s0=0, sd=[[1, 2], [1, 2]]),
           dict(x0=9, xd=[[-2, 2], [-1, 2]],
                t0=2, td=[[2, 2], [1, 2]],
                s0=1, sd=[[0, 2], [-1, 2]])],
    ),
    6: dict(  # pairs (B,F,E,C); split into the {F,E} half (slots 6..9,
        # covered by the first x chunk, so it starts before slots 12,13
        # land) and the {B,C} half (slots 2,3,12,13)
        t=[dict(x0=6, xd=[[2, 2], [1, 2]], t0=2, td=[[2, 2], [1, 2]]),
           dict(x0=2, xd=[[10, 2], [1, 2]], t0=0, td=[[6, 2], [1, 2]])],
        u=[dict(x0=8, xd=[[-2, 2], [1, 2]], t0=2, td=[[2, 2], [1, 2]],
                s0=1, sd=[[-1, 2], [1, 2]]),
           dict(x0=12, xd=[[-10, 2], [1, 2]], t0=0, td=[[6, 2], [1, 2]],
                s0=0, sd=[[1, 2], [1, 2]])],
    ),
}


def _iter_idx(dims):
    import itertools
    return itertools.product(*[range(c) for (_, c) in dims])


def _off(dims, idx):
    return sum(s * i for (s, _), i in zip(dims, idx))


def _verify_plane_ops(cayley):
    """Symbolically apply the descriptor index arithmetic for one position:
    out[comp] = c2*x[tcomp] + seqsign*s2*x[ucomp], and check it equals the
    Cayley-derived Givens stage for every plane.  Raises on mismatch."""
    for m in PLANE_BLADES:
        ops = _PLANE_OPS[m]
        tmap, umap, usgn, amap = {}, {}, {}, {}
        for sub in ops["t"]:
            for idx in _iter_idx(sub["xd"]):
                tp = sub["t0"] + _off(sub["td"], idx)
                sp = sub["x0"] + _off(sub["xd"], idx)
                assert tp not in tmap, (m, tp)
                tmap[tp] = sp
        for sub in ops.get("a", ops["t"]):
            for idx in _iter_idx(sub["xd"]):
                tp = sub["t0"] + _off(sub["td"], idx)
                sp = sub["x0"] + _off(sub["xd"], idx)
                assert tp not in amap, (m, tp)
                amap[tp] = sp
        assert amap == tmap, (m, amap, tmap)
        seq = _PLANE_SEQ[m]
        for sub in ops["u"]:
            for idx in _iter_idx(sub["xd"]):
                tp = sub["t0"] + _off(sub["td"], idx)
                sp = sub["x0"] + _off(sub["xd"], idx)
                blk = sub["s0"] + _off(list(zip([s for s, _ in sub["sd"]],
                                                [c for _, c in sub["xd"]])), idx)
                assert 0 <= blk < len(seq), (m, blk)
                assert tp not in umap, (m, tp)
                umap[tp] = sp
                usgn[tp] = seq[blk]
        assert sorted(tmap) == sorted(umap) == list(range(8)), m
        for tp in range(8):
            a = COMP_OF_SLOT[tmap[tp]]
            b = a ^ m
            assert COMP_OF_SLOT[umap[tp]] == b, (m, tp, COMP_OF_SLOT[umap[tp]], b)
            tau = float(cayley[a, m, b])
            assert usgn[tp] == tau, (m, tp, usgn[tp], tau)
        # every rotating slot pair must be read/written exactly once
        rot = sorted(tmap.values())
        expect = sorted(s for s in range(16)
                        if bin(COMP_OF_SLOT[s] & m).count("1") == 1)
        assert rot == expect, (m, rot, expect)


def _ap_with_dims(base_ap, extra_off, dims):
    ap = [list(base_ap.ap[0])] + [list(d) for d in dims]
    return bass.AP(base_ap.tensor, base_ap.offset + extra_off, ap)


def _merged(sub):
    md = all(d[-1] == [1, 2] for d in (sub["xd"], sub["td"]))
    if "sd" in sub:
        md = md and sub["sd"][-1][0] == 1
    return md


def _el(slotdims, merged):
    if merged:
        return [[s * J, n] for s, n in slotdims[:-1]] + [[1, 2 * J]]
    return [[s * J, n] for s, n in slotdims] + [[1, J]]


def _el_blk(blksteps, counts, merged):
    if merged:
        return [[s * J, n] for (s, _), (_, n) in
                zip(blksteps[:-1], counts[:-1])] + [[1, 2 * J]]
    return [[s * J, n] for (s, _), (_, n) in zip(blksteps, counts)] + [[1, J]]


def _cdims(sub, merged):
    if merged:
        return [[0, n] for _, n in sub["td"][:-1]] + [[1, 2 * J]]
    return [[0, n] for _, n in sub["td"]] + [[1, J]]


def _build_program(freqs, coefs):
    nc = bacc.Bacc("TRN2", target_bir_lowering=False, debug=False,
                   enable_asserts=False, num_devices=NCORES)
    x_d = nc.dram_tensor("x", [P, 16 * J], F16, kind="ExternalInput")
    pos_d = nc.dram_tensor("pos", [P, J], I32, kind="ExternalInput")
    out_d = nc.dram_tensor("out", [P, 16 * J], F16, kind="ExternalOutput")

    SIN = mybir.ActivationFunctionType.Sin
    plane_i = {m: PLANE_BLADES.index(m) for m in STAGE_ORDER}

    with tile.TileContext(nc) as tc:
        with tc.tile_pool(name="const", bufs=1) as cpool, \
             tc.tile_pool(name="ang", bufs=2) as apool, \
             tc.tile_pool(name="tmp", bufs=2) as tpool:

            X = cpool.tile([P, 16 * J], F16)
            Pp = cpool.tile([P, J], I32)
            nc.sync.dma_start(Pp[:], pos_d[:])
            # one wide chunk covers everything stage m=6's {F,E} half (and
            # its {B,C} half's B side) reads; 4KB-row descriptors keep the
            # DMA engines rate-efficient even though [4J,6J) is only needed
            # by stage 2
            nc.sync.dma_start(X[:, 2 * J:10 * J], x_d[:, 2 * J:10 * J])

            # ---- per-plane angle pipelines + fp16 sin/cos tables ----
            # A = pos*(f*c) and K = round(A/2pi) on ScalarE (scale/bias folds
            # the magic-round); only the Cody-Waite cascade runs on DVE.
            COPY = mybir.ActivationFunctionType.Copy
            IDENT = mybir.ActivationFunctionType.Identity
            ABS = mybir.ActivationFunctionType.Abs
            CB = {}
            for ci, v in enumerate((MAGIC, -MAGIC, HALF_PI)):
                cb = cpool.tile([P, 1], F32, name=f"bias{ci}")
                nc.gpsimd.memset(cb[:], v)
                CB[v] = cb
            # dummy Sin so the one act-table set holding Copy+Identity+Sin
            # (trig_and_small) is loaded ONCE, before the angle chains --
            # otherwise the first Copy loads a sin-less set and the first
            # real Sin pays a 1.3us table reload on the critical path.
            warm = cpool.tile([P, 1], F32, name="warm")
            nc.scalar.activation(warm[:], CB[HALF_PI][:], SIN)
            Cd, SX = {}, {}
            for si, m in enumerate(STAGE_ORDER):
                i = plane_i[m]
                fc = float(np.float32(np.float32(freqs[i]) *
                                      np.float32(coefs[i])))
                A = apool.tile([P, J], F32, tag="a")
                K1 = apool.tile([P, J], F32, tag="k1")
                K = apool.tile([P, J], F32, tag="k")
                R = apool.tile([P, J], F32, tag="r")
                RC = apool.tile([P, J], F32, tag="rc")
                # A = pos*(f*c) and K = round(A/2pi) on ScalarE (scale/bias
                # folds the magic round); Cody-Waite on DVE.  The first
                # plane's chain gates the whole stage pipeline: pin it to
                # the front of every engine queue.
                def prio(first=si == 0):
                    return (tc.high_priority() if first
                            else contextlib.nullcontext())
                with prio():
                    if si == 0:
                        # K and |R| on the (idle) DVE so the ScalarE queue
                        # never stalls this chain behind later planes' ops
                        nc.scalar.activation(K1[:], Pp[:], IDENT,
                                             bias=CB[MAGIC][:],
                                             scale=float(np.float32(fc) *
                                                         INV_2PI))
                        nc.scalar.activation(A[:], Pp[:], COPY, scale=fc)
                        nc.vector.tensor_scalar_add(K[:], K1[:], -MAGIC)
                        nc.vector.cody_waite_cascade(R[:], A[:], K[:],
                                                     CW1, CW2, CW3)
                        # cos arg via the one-op wrap on the still-idle DVE:
                        # Cd then has no ScalarE prerequisite, so it can't
                        # lose its queue slot to the (ready-earlier) SX sins
                        nc.vector.add_range_wrap(RC[:], R[:], HALF_PI,
                                                 PI_F, TWO_PI_F)
                    else:
                        nc.scalar.activation(A[:], Pp[:], COPY, scale=fc)
                        nc.scalar.activation(K1[:], Pp[:], IDENT,
                                             bias=CB[MAGIC][:],
                                             scale=float(np.float32(fc) *
                                                         INV_2PI))
                        if si == 1:
                            # m=9's K/CW fit in the DVE's pre-stage idle gap
                            nc.vector.tensor_scalar_add(K[:], K1[:], -MAGIC)
                        else:
                            # m=5/m=3's chains resolve mid-window: keep their
                            # K on ScalarE so the DVE only runs the cascade
                            nc.scalar.activation(K[:], K1[:], IDENT,
                                                 bias=CB[-MAGIC][:])
                        nc.vector.cody_waite_cascade(R[:], A[:], K[:],
                                                     CW1, CW2, CW3)

                seq = _PLANE_SEQ[m]
                Cd[m] = cpool.tile([P, 2 * J], F16, name=f"cd{m}",
                                   tag=f"c{m}")
                SX[m] = cpool.tile([P, len(seq) * J], F16, name=f"sx{m}",
                                   tag=f"s{m}")
                with prio():
                    if si == 0:
                        nc.scalar.activation(
                            _ap_with_dims(Cd[m][:], 0, [[J, 2], [1, J]]),
                            _ap_with_dims(RC[:], 0, [[0, 2], [1, J]]), SIN)
                    else:
                        # cos arg as sin(pi/2 - |R|): stays in Sin's domain
                        nc.scalar.activation(RC[:], R[:], ABS)
                        nc.scalar.activation(
                            _ap_with_dims(Cd[m][:], 0, [[J, 2], [1, J]]),
                            _ap_with_dims(RC[:], 0, [[0, 2], [1, J]]), SIN,
                            scale=-1.0, bias=CB[HALF_PI][:])
                # the sin tables gate only the U op (one T later than Cd):
                # left outside the priority block so their completion sems
                # don't batch with Cd's and stall the first T
                for sgn in (1.0, -1.0):
                    blks = [b for b, s in enumerate(seq) if s == sgn]
                    if len(blks) == 1:
                        od = [[1, J]]
                    else:
                        od = [[(blks[1] - blks[0]) * J, len(blks)],
                              [1, J]]
                    nc.scalar.activation(
                        _ap_with_dims(SX[m][:], blks[0] * J, od),
                        _ap_with_dims(R[:], 0,
                                      [[0, len(blks)], [1, J]][-len(od):]),
                        SIN, scale=sgn)

            # slots 12,13 (needed by m=6's second half), slots 10,11 (2nd
            # stage), slots 0,1 (3rd stage) and the slots-14,15 DRAM->DRAM
            # passthrough (never rotated).  Emitted after the angle chains
            # so their completion sems don't batch with the first chunk's,
            # which alone gates the {F,E} half of stage m=6.
            nc.sync.dma_start(X[:, 12 * J:14 * J], x_d[:, 12 * J:14 * J])
            nc.sync.dma_start(X[:, 10 * J:12 * J], x_d[:, 10 * J:12 * J])
            nc.sync.dma_start(X[:, :2 * J], x_d[:, :2 * J])
            nc.sync.dma_start(out_d[:, 14 * J:], x_d[:, 14 * J:])

            # ---- Givens stages (innermost rotor first) ----
            for si, m in enumerate(STAGE_ORDER):
                ops = _PLANE_OPS[m]
                T = tpool.tile([P, 8 * J], F16, tag="t")
                U = tpool.tile([P, 8 * J], F16, tag="u")
                for sub in ops["t"]:
                    md = _merged(sub)
                    nc.vector.tensor_mul(
                        _ap_with_dims(T[:], sub["t0"] * J, _el(sub["td"], md)),
                        _ap_with_dims(X[:], sub["x0"] * J, _el(sub["xd"], md)),
                        _ap_with_dims(Cd[m][:], 0, _cdims(sub, md)))
                for sub in ops["u"]:
                    md = _merged(sub)
                    nc.vector.tensor_mul(
                        _ap_with_dims(U[:], sub["t0"] * J, _el(sub["td"], md)),
                        _ap_with_dims(X[:], sub["x0"] * J, _el(sub["xd"], md)),
                        _ap_with_dims(SX[m][:], sub["s0"] * J,
                                      _el_blk(sub["sd"], sub["xd"], md)))
                for sub in ops.get("a", ops["t"]):
                    md = _merged(sub)
                    nc.vector.tensor_add(
                        _ap_with_dims(X[:], sub["x0"] * J, _el(sub["xd"], md)),
                        _ap_with_dims(T[:], sub["t0"] * J, _el(sub["td"], md)),
                        _ap_with_dims(U[:], sub["t0"] * J, _el(sub["td"], md)))
                if si == 1:
                    # pair D (slots 10,11) is final after the 2nd stage (m=9)
                    nc.sync.dma_start(out_d[:, 10 * J:12 * J],
                                      X[:, 10 * J:12 * J])
                if si == 2:
                    # pairs E,C (slots 8,9 / 12,13) final after 3rd stage
                    nc.sync.dma_start(out_d[:, 8 * J:10 * J],
                                      X[:, 8 * J:10 * J])
                    nc.sync.dma_start(out_d[:, 12 * J:14 * J],
                                      X[:, 12 * J:14 * J])
            # final stage (m=3) ADD is split 3+1 pairs; each part's slots DMA
            # out as soon as its ADD lands.  Two chunks, not three: every
            # extra trigger costs ~0.6us serially on the sync sequencer at
            # the very tail of the kernel.
            nc.sync.dma_start(out_d[:, :6 * J], X[:, :6 * J])
            nc.sync.dma_start(out_d[:, 6 * J:8 * J], X[:, 6 * J:8 * J])

    nc.compile()
    return nc


_PROGRAM_CACHE = {}


def _get_program(freqs, coefs):
    key = (tuple(freqs), tuple(coefs))
    if key not in _PROGRAM_CACHE:
        _PROGRAM_CACHE[key] = _build_program(freqs, coefs)
    return _PROGRAM_CACHE[key]


def _derive_params(inputs):
    coefs = [float(np.asarray(inputs[c], dtype=np.float32).reshape(MV)[b])
             for c, b in zip(("bx", "by", "bz", "bw"), PLANE_BLADES)]
    theta = np.asarray(inputs["theta"], dtype=np.float32)
    freqs = [float(theta.reshape(MAX_LEN, 4)[1, i]) for i in range(4)]
    return freqs, coefs


def _core_input(x, pos, g):
    xg = np.asarray(x[g * ROWS_PER_CORE:(g + 1) * ROWS_PER_CORE],
                    dtype=np.float32).reshape(P, J, MV)
    planar = xg[:, :, COMP_OF_SLOT].transpose(0, 2, 1)
    pg = np.clip(pos[g * ROWS_PER_CORE:(g + 1) * ROWS_PER_CORE],
                 0, MAX_LEN - 1).astype(np.int32).reshape(P, J)
    return {"x": np.ascontiguousarray(planar.astype(np.float16)
                                      ).reshape(P, 16 * J),
            "pos": np.ascontiguousarray(pg)}


def _core_output(res_g):
    r = np.asarray(res_g).reshape(P, 16, J).transpose(0, 2, 1)
    return r[:, :, SLOT_OF_COMP].astype(np.float32).reshape(
        ROWS_PER_CORE, L, MV)


def kernel(x, pos, bx, by, bz, bw, theta, cayley, biv_mask, scalar_mask):
    x = np.asarray(x, dtype=np.float32)
    pos = np.asarray(pos)
    theta = np.asarray(theta, dtype=np.float32)
    cayley = np.asarray(cayley, dtype=np.float32)

    assert x.shape == (B, L, MV) and pos.shape == (B, L)

    freqs, coefs = _derive_params(
        dict(bx=bx, by=by, bz=bz, bw=bw, theta=theta))
    th_check = np.arange(MAX_LEN, dtype=np.float32)[:, None] * \
        np.asarray(freqs, dtype=np.float32)[None, :]
    assert np.array_equal(th_check, theta.reshape(MAX_LEN, 4)), \
        "theta table is not linear in position; kernel assumption violated"

    _verify_plane_ops(cayley)

    nc = _get_program(freqs, coefs)

    in_maps = [_core_input(x, pos, g) for g in range(NCORES)]
    res = run_bass_kernel_spmd(nc, in_maps, core_ids=list(range(NCORES)))
    out = np.empty((B, L, MV), dtype=np.float32)
    for g in range(NCORES):
        out[g * ROWS_PER_CORE:(g + 1) * ROWS_PER_CORE] = \
            _core_output(res.results[g]["out"])
    return out
